# revision 8
# baseline (speedup 1.0000x reference)
"""Trainium2 Bass kernel for nn_Encoding (dense transformer block with
inter-attention + gated fusion), data-parallel over batch on 8 NeuronCores.

Reference math per batch b (P: [n, d], weights small):
  wa, wb, wc = split(w_itr_att)
  A[i,j]   = P[i].wb + P[j].wa + sum_d P[i,d]*wc[d]*P[j,d]
  SA       = softmax_j(A)
  itr      = SA @ P
  Pc       = [P, itr]
  z = tanh(Pc@w1+b1); r = sig(Pc@w2+b2); f = sig(Pc@w3+b3)
  out      = r*P + f*z

Key algebra used here:
  - exp(P[i].wb) cancels between softmax numerator and denominator, so the
    wb term is dropped entirely.
  - Scores are computed TRANSPOSED: At[j,i] = A[i,j], so the row term
    P[j].wa is a per-partition bias folded into the exp() activation and
    both the numerator matmul (P^T @ T) and denominator matmul
    (ones^T @ T) consume T=exp(At) in its natural layout with j on
    partitions (contraction axis) -- no transpose of the big [n,n] matrix.
  - sigmoid(x) = 0.5 + 0.5*tanh(0.5*x) keeps all activations within the
    exp/tanh ACT table set (no table switches).
All matmuls run in float32r (fp32 storage, ~1e-4 matmul relative error,
1 cycle/row on the PE when N>=256).
"""
from contextlib import ExitStack

import numpy as np

import concourse.bass as bass
import concourse.mybir as mybir
import concourse.tile as tile
import concourse.tile_sem_assignment as tsa
from concourse import bacc
from concourse.bass_utils import run_bass_kernel_spmd
from concourse.masks import make_identity

# All HWDGE DMAs here are issued from the single SP sequencer (one physical
# FIFO ring -> in-order completion), so one completion semaphore suffices and
# keeps per-instruction sync-wait counts low.
tsa.NUM_HWDGE_SEMS = 1

B, N, D = 32, 1024, 128
NCORES = 8
BPC = B // NCORES          # batches per core
NB = N // 128              # 128-row blocks per batch
f32 = mybir.dt.float32
f32r = mybir.dt.float32r
Exp = mybir.ActivationFunctionType.Exp
Tanh = mybir.ActivationFunctionType.Tanh


def _body(nc, tc, ctx):
    P = nc.dram_tensor("P", [BPC, N, D], f32, kind="ExternalInput")
    w_att = nc.dram_tensor("w_itr_att", [3 * D], f32, kind="ExternalInput")
    w1 = nc.dram_tensor("w1", [2 * D, D], f32, kind="ExternalInput")
    w2 = nc.dram_tensor("w2", [2 * D, D], f32, kind="ExternalInput")
    w3 = nc.dram_tensor("w3", [2 * D, D], f32, kind="ExternalInput")
    b1 = nc.dram_tensor("b1", [D], f32, kind="ExternalInput")
    b2 = nc.dram_tensor("b2", [D], f32, kind="ExternalInput")
    b3 = nc.dram_tensor("b3", [D], f32, kind="ExternalInput")
    out = nc.dram_tensor("out", [BPC, N, D], f32, kind="ExternalOutput")

    singles = ctx.enter_context(tc.tile_pool(name="singles", bufs=1))
    work = ctx.enter_context(tc.tile_pool(name="work", bufs=2))
    big = ctx.enter_context(tc.tile_pool(name="big", bufs=1))
    ps_at = ctx.enter_context(tc.tile_pool(name="ps_at", bufs=2, space="PSUM"))
    ps_itr = ctx.enter_context(tc.tile_pool(name="ps_itr", bufs=2, space="PSUM"))
    ps_dv = ctx.enter_context(tc.tile_pool(name="ps_dv", bufs=2, space="PSUM"))

    # ---- constants ----
    ident = singles.tile([128, 128], f32)
    make_identity(nc, ident)
    ident_r = singles.tile([128, 128], f32r)
    nc.vector.tensor_copy(ident_r, ident)

    ones_f = singles.tile([128, 1], f32)
    nc.vector.memset(ones_f, 1.0)
    ones_rf = singles.tile([1, 128], f32)
    nc.vector.memset(ones_rf, 1.0)
    ones_col = singles.tile([128, 1], f32r)    # lhsT for denominator matmul
    nc.vector.tensor_copy(ones_col, ones_f)
    ones_row = singles.tile([1, 128], f32r)    # lhsT for broadcast matmuls
    nc.vector.tensor_copy(ones_row, ones_rf)

    # wa / wc columns ([128,1]); wb is mathematically irrelevant (cancels).
    watt_sb = singles.tile([128, 3], f32)
    nc.sync.dma_start(out=watt_sb, in_=w_att.rearrange("(c p) -> p c", p=128))
    wa_col = singles.tile([128, 2], f32r)   # duplicated: fp32r matmul needs N>=2
    nc.vector.tensor_copy(wa_col[:, 0:1], watt_sb[:, 0:1])
    nc.vector.tensor_copy(wa_col[:, 1:2], watt_sb[:, 0:1])
    wc_col = singles.tile([128, 1], f32)
    nc.vector.tensor_copy(wc_col, watt_sb[:, 2:3])

    # Gate weights: Wtop = rows 0:128 of [w1|w2|w3], Wbot = rows 128:256.
    wstage = singles.tile([128, 2, 3, 128], f32)
    for gi, w in enumerate((w1, w2, w3)):
        nc.sync.dma_start(out=wstage[:, 0, gi, :], in_=w[0:128, :])
        nc.sync.dma_start(out=wstage[:, 1, gi, :], in_=w[128:256, :])
    w_top = singles.tile([128, 384], f32r)
    w_bot = singles.tile([128, 384], f32r)
    nc.vector.tensor_copy(w_top, wstage[:, 0, :, :])
    nc.vector.tensor_copy(w_bot, wstage[:, 1, :, :])

    bstage = singles.tile([1, 3, 128], f32)
    for gi, bvec in enumerate((b1, b2, b3)):
        nc.sync.dma_start(out=bstage[:, gi, :],
                          in_=bvec.rearrange("(o p) -> o p", o=1))
    bcat = singles.tile([1, 384], f32r)
    nc.vector.tensor_copy(bcat, bstage)

    for bi in range(BPC):
        # ---- phase A: load P, build P^T (via PE transpose), Pwc^T, v ----
        pn = work.tile([128, NB, 128], f32, tag="pn")
        nc.sync.dma_start(out=pn, in_=P[bi].rearrange("(t p) d -> p t d", p=128))
        pn_r = work.tile([128, NB, 128], f32r, tag="pn_r")
        nc.gpsimd.tensor_copy(pn_r, pn)

        pt_r = work.tile([128, NB, 128], f32r, tag="pt_r")   # [d, n]
        for half in range(2):
            tp_ps = ps_itr.tile([128, 512], f32r, tag="itr")
            for q in range(4):
                jb = half * 4 + q
                nc.tensor.transpose(tp_ps[:, q * 128:(q + 1) * 128],
                                    pn_r[:, jb, :], ident_r)
            nc.vector.tensor_copy(
                pt_r[:, half * 4:(half + 1) * 4, :], tp_ps)

        pwct_r = work.tile([128, NB, 128], f32r, tag="pwct_r")  # wc[d]*P^T
        nc.vector.tensor_scalar_mul(pwct_r, pt_r, wc_col)

        # v[j] = P[j].wa computed per j-block into [128, NB, 2] (col 0 used)
        v_ps = ps_dv.tile([128, NB, 2], f32, tag="dv")
        for jb in range(NB):
            nc.tensor.matmul(v_ps[:, jb, :], pt_r[:, jb, :], wa_col,
                             start=True, stop=True)
        v_sb = work.tile([128, NB, 2], f32, tag="v_sb")
        nc.vector.tensor_copy(v_sb, v_ps)

        # ---- phase B: scores At[j,i] + exp -> T ----
        st = big.tile([128, NB, N], f32r, tag="st")
        for jb in range(NB):
            at_ps = ps_at.tile([128, 1024], f32, tag="at")
            nc.tensor.matmul(at_ps[:, 0:512], pt_r[:, jb, :],
                             pwct_r[:, 0:4, :], start=True, stop=True)
            nc.tensor.matmul(at_ps[:, 512:1024], pt_r[:, jb, :],
                             pwct_r[:, 4:8, :], start=True, stop=True)
            nc.scalar.activation(st[:, jb, :], at_ps, Exp,
                                 bias=v_sb[:, jb, 0:1])

        # ---- phase C: numerator/denominator + normalize ----
        itrt_r = work.tile([128, NB, 128], f32r, tag="itrt_r")  # itr^T [d, n]
        for c in range(2):
            cs = slice(c * 512, (c + 1) * 512)
            itr_ps = ps_itr.tile([128, 512], f32, tag="itr")
            den_ps = ps_dv.tile([1, 512], f32, tag="dv")
            for jb in range(NB):
                nc.tensor.matmul(itr_ps, pn_r[:, jb, :], st[:, jb, cs],
                                 start=(jb == 0), stop=(jb == NB - 1))
            for jb in range(NB):
                nc.tensor.matmul(den_ps, ones_col, st[:, jb, cs],
                                 start=(jb == 0), stop=(jb == NB - 1))
            recip = work.tile([1, 512], f32r, tag="recip")
            with nc.allow_low_precision(reason="feeds fp32r matmul"):
                nc.vector.reciprocal(recip, den_ps)
            bc_ps = ps_dv.tile([128, 512], f32, tag="dv")
            nc.tensor.matmul(bc_ps, ones_row, recip, start=True, stop=True)
            bc_sb = work.tile([128, 512], f32, tag="bc_sb")
            nc.vector.tensor_copy(bc_sb, bc_ps)
            with nc.allow_low_precision(reason="fp32r itr weights"):
                nc.vector.tensor_mul(itrt_r[:, c * 4:(c + 1) * 4, :],
                                     itr_ps, bc_sb)

        # ---- phase D: gates ----
        gcat = work.tile([128, NB, 384], f32, tag="gcat")
        for ib in range(NB):
            g_ps = ps_dv.tile([128, 384], f32, tag="dv")
            nc.tensor.matmul(g_ps, pt_r[:, ib, :], w_top, start=True, stop=False)
            nc.tensor.matmul(g_ps, itrt_r[:, ib, :], w_bot, start=False, stop=False)
            nc.tensor.matmul(g_ps, ones_row, bcat, start=False, stop=True)
            nc.vector.tensor_copy(gcat[:, ib, :], g_ps)

        z_t = work.tile([128, NB, 128], f32, tag="z_t")
        nc.scalar.activation(z_t, gcat[:, :, 0:128], Tanh)
        rf_t = work.tile([128, NB, 256], f32, tag="rf_t")
        nc.scalar.activation(rf_t, gcat[:, :, 128:384], Tanh, scale=0.5)
        # r = 0.5 + 0.5*tanh(0.5 x), f likewise
        rf_a = work.tile([128, NB, 256], f32, tag="rf_a")
        nc.gpsimd.tensor_scalar(rf_a, rf_t, 0.5, 0.5,
                                mybir.AluOpType.mult, mybir.AluOpType.add)

        m1 = work.tile([128, NB, 128], f32, tag="m1")
        nc.gpsimd.tensor_mul(m1, rf_a[:, :, 0:128], pn)       # r * P
        m2 = work.tile([128, NB, 128], f32, tag="m2")
        nc.vector.tensor_mul(m2, rf_a[:, :, 128:256], z_t)    # f * z
        out_sb = work.tile([128, NB, 128], f32, tag="out_sb")
        nc.vector.tensor_add(out_sb, m1, m2)

        nc.sync.dma_start(out=out[bi].rearrange("(t p) d -> p t d", p=128),
                          in_=out_sb)


_NC_CACHE = {}


def _get_nc():
    if "nc" not in _NC_CACHE:
        nc = bacc.Bacc(None)
        with tile.TileContext(nc) as tc:
            with ExitStack() as ctx:
                _body(nc, tc, ctx)
        nc.finalize()
        _NC_CACHE["nc"] = nc
    return _NC_CACHE["nc"]


def _run(inputs, **kw):
    nc = _get_nc()
    in_maps = []
    for c in range(NCORES):
        m = {
            "P": np.ascontiguousarray(inputs["P"][c * BPC:(c + 1) * BPC]),
            "w_itr_att": np.asarray(inputs["w_itr_att"]),
            "w1": np.asarray(inputs["w1"]),
            "w2": np.asarray(inputs["w2"]),
            "w3": np.asarray(inputs["w3"]),
            "b1": np.asarray(inputs["b1"]),
            "b2": np.asarray(inputs["b2"]),
            "b3": np.asarray(inputs["b3"]),
        }
        in_maps.append({k: np.asarray(v, dtype=np.float32) for k, v in m.items()})
    res = run_bass_kernel_spmd(nc, in_maps, core_ids=list(range(NCORES)), **kw)
    outp = np.concatenate([r["out"] for r in res.results], axis=0)
    return outp.astype(np.float32), res


def kernel(**inputs):
    out, _ = _run(inputs)
    return out


# revision 10
# speedup vs baseline: 1.0283x; 1.0283x over previous
"""Trainium2 Bass kernel for nn_Encoding (dense transformer block with
inter-attention + gated fusion), data-parallel over batch on 8 NeuronCores.

Reference math per batch b (P: [n, d], weights small):
  wa, wb, wc = split(w_itr_att)
  A[i,j]   = P[i].wb + P[j].wa + sum_d P[i,d]*wc[d]*P[j,d]
  SA       = softmax_j(A)
  itr      = SA @ P
  Pc       = [P, itr]
  z = tanh(Pc@w1+b1); r = sig(Pc@w2+b2); f = sig(Pc@w3+b3)
  out      = r*P + f*z

Key algebra used here:
  - exp(P[i].wb) cancels between softmax numerator and denominator, so the
    wb term is dropped entirely.
  - Scores are computed TRANSPOSED: At[j,i] = A[i,j], so the row term
    P[j].wa is a per-partition bias folded into the exp() activation and
    both the numerator matmul (P^T @ T) and denominator matmul
    (ones^T @ T) consume T=exp(At) in its natural layout with j on
    partitions (contraction axis) -- no transpose of the big [n,n] matrix.
  - sigmoid(x) = 0.5 + 0.5*tanh(0.5*x) keeps all activations within the
    exp/tanh ACT table set (no table switches).
All matmuls run in float32r (fp32 storage, ~1e-4 matmul relative error,
1 cycle/row on the PE when N>=256).
"""
from contextlib import ExitStack

import numpy as np

import concourse.bass as bass
import concourse.mybir as mybir
import concourse.tile as tile
import concourse.tile_sem_assignment as tsa
from concourse import bacc
from concourse.bass_utils import run_bass_kernel_spmd
from concourse.masks import make_identity

# All HWDGE DMAs here are issued from the single SP sequencer (one physical
# FIFO ring -> in-order completion), so one completion semaphore suffices and
# keeps per-instruction sync-wait counts low.
tsa.NUM_HWDGE_SEMS = 1

B, N, D = 32, 1024, 128
NCORES = 8
BPC = B // NCORES          # batches per core
NB = N // 128              # 128-row blocks per batch
f32 = mybir.dt.float32
f32r = mybir.dt.float32r
Exp = mybir.ActivationFunctionType.Exp
Tanh = mybir.ActivationFunctionType.Tanh


def _body(nc, tc, ctx):
    P = nc.dram_tensor("P", [BPC, N, D], f32, kind="ExternalInput")
    w_att = nc.dram_tensor("w_itr_att", [3 * D], f32, kind="ExternalInput")
    w1 = nc.dram_tensor("w1", [2 * D, D], f32, kind="ExternalInput")
    w2 = nc.dram_tensor("w2", [2 * D, D], f32, kind="ExternalInput")
    w3 = nc.dram_tensor("w3", [2 * D, D], f32, kind="ExternalInput")
    b1 = nc.dram_tensor("b1", [D], f32, kind="ExternalInput")
    b2 = nc.dram_tensor("b2", [D], f32, kind="ExternalInput")
    b3 = nc.dram_tensor("b3", [D], f32, kind="ExternalInput")
    out = nc.dram_tensor("out", [BPC, N, D], f32, kind="ExternalOutput")

    singles = ctx.enter_context(tc.tile_pool(name="singles", bufs=1))
    work = ctx.enter_context(tc.tile_pool(name="work", bufs=2))
    big = ctx.enter_context(tc.tile_pool(name="big", bufs=1))
    ps_at = ctx.enter_context(tc.tile_pool(name="ps_at", bufs=2, space="PSUM"))
    ps_itr = ctx.enter_context(tc.tile_pool(name="ps_itr", bufs=2, space="PSUM"))
    ps_dv = ctx.enter_context(tc.tile_pool(name="ps_dv", bufs=2, space="PSUM"))

    # ---- constants ----
    ident = singles.tile([128, 128], f32)
    make_identity(nc, ident)
    ident_r = singles.tile([128, 128], f32r)
    nc.vector.tensor_copy(ident_r, ident)

    ones_f = singles.tile([128, 1], f32)
    nc.vector.memset(ones_f, 1.0)
    ones_rf = singles.tile([1, 128], f32)
    nc.vector.memset(ones_rf, 1.0)
    ones_col = singles.tile([128, 1], f32r)    # lhsT for denominator matmul
    nc.vector.tensor_copy(ones_col, ones_f)
    ones_row = singles.tile([1, 128], f32r)    # lhsT for broadcast matmuls
    nc.vector.tensor_copy(ones_row, ones_rf)

    # wa / wc columns ([128,1]); wb is mathematically irrelevant (cancels).
    watt_sb = singles.tile([128, 3], f32)
    nc.sync.dma_start(out=watt_sb, in_=w_att.rearrange("(c p) -> p c", p=128))
    wa_col = singles.tile([128, 2], f32r)   # duplicated: fp32r matmul needs N>=2
    nc.vector.tensor_copy(wa_col[:, 0:1], watt_sb[:, 0:1])
    nc.vector.tensor_copy(wa_col[:, 1:2], watt_sb[:, 0:1])
    wc_col = singles.tile([128, 1], f32)
    nc.vector.tensor_copy(wc_col, watt_sb[:, 2:3])

    # Gate weights: Wtop = rows 0:128 of [w1|w2|w3], Wbot = rows 128:256.
    wstage = singles.tile([128, 2, 3, 128], f32)
    for gi, w in enumerate((w1, w2, w3)):
        nc.sync.dma_start(out=wstage[:, 0, gi, :], in_=w[0:128, :])
        nc.sync.dma_start(out=wstage[:, 1, gi, :], in_=w[128:256, :])
    w_top = singles.tile([128, 384], f32r)
    w_bot = singles.tile([128, 384], f32r)
    nc.vector.tensor_copy(w_top, wstage[:, 0, :, :])
    nc.vector.tensor_copy(w_bot, wstage[:, 1, :, :])

    bstage = singles.tile([1, 3, 128], f32)
    for gi, bvec in enumerate((b1, b2, b3)):
        nc.sync.dma_start(out=bstage[:, gi, :],
                          in_=bvec.rearrange("(o p) -> o p", o=1))
    bcat = singles.tile([1, 384], f32r)
    nc.vector.tensor_copy(bcat, bstage)

    for bi in range(BPC):
        # ---- phase A: load P, build P^T (via PE transpose), Pwc^T, v ----
        pn = work.tile([128, NB, 128], f32, tag="pn")
        nc.sync.dma_start(out=pn, in_=P[bi].rearrange("(t p) d -> p t d", p=128))
        pn_r = work.tile([128, NB, 128], f32r, tag="pn_r")
        nc.gpsimd.tensor_copy(pn_r, pn)

        pt_r = work.tile([128, NB, 128], f32r, tag="pt_r")   # [d, n]
        for half in range(2):
            tp_ps = ps_itr.tile([128, 512], f32r, tag="itr")
            for q in range(4):
                jb = half * 4 + q
                nc.tensor.transpose(tp_ps[:, q * 128:(q + 1) * 128],
                                    pn_r[:, jb, :], ident_r)
            nc.vector.tensor_copy(
                pt_r[:, half * 4:(half + 1) * 4, :], tp_ps)

        pwct_r = work.tile([128, NB, 128], f32r, tag="pwct_r")  # wc[d]*P^T
        nc.vector.tensor_scalar_mul(pwct_r, pt_r, wc_col)

        # v[j] = P[j].wa computed per j-block into [128, NB, 2] (col 0 used)
        v_ps = ps_dv.tile([128, NB, 2], f32, tag="dv")
        for jb in range(NB):
            nc.tensor.matmul(v_ps[:, jb, :], pt_r[:, jb, :], wa_col,
                             start=True, stop=True)
        v_sb = work.tile([128, NB, 2], f32, tag="v_sb")
        nc.vector.tensor_copy(v_sb, v_ps)

        # ---- phase B: scores At[j,i] + exp -> T ----
        st = big.tile([128, NB, N], f32r, tag="st")
        for jb in range(NB):
            at_ps = ps_at.tile([128, 1024], f32, tag="at")
            nc.tensor.matmul(at_ps[:, 0:512], pt_r[:, jb, :],
                             pwct_r[:, 0:4, :], start=True, stop=True)
            nc.tensor.matmul(at_ps[:, 512:1024], pt_r[:, jb, :],
                             pwct_r[:, 4:8, :], start=True, stop=True)
            nc.scalar.activation(st[:, jb, :], at_ps, Exp,
                                 bias=v_sb[:, jb, 0:1])

        # ---- phase C: numerator/denominator + normalize ----
        itrt_r = work.tile([128, NB, 128], f32r, tag="itrt_r")  # itr^T [d, n]
        for c in range(2):
            cs = slice(c * 512, (c + 1) * 512)
            itr_ps = ps_itr.tile([128, 512], f32, tag="itr")
            den_ps = ps_dv.tile([1, 512], f32, tag="dv")
            for jb in range(NB):
                nc.tensor.matmul(itr_ps, pn_r[:, jb, :], st[:, jb, cs],
                                 start=(jb == 0), stop=(jb == NB - 1))
            for jb in range(NB):
                nc.tensor.matmul(den_ps, ones_col, st[:, jb, cs],
                                 start=(jb == 0), stop=(jb == NB - 1))
            # broadcast raw denominator to all partitions, then reciprocal
            # on [128,512] (a [1,512] reciprocal runs on one DVE lane: ~3.3us)
            den_row = work.tile([1, 512], f32r, tag="den_row")
            with nc.allow_low_precision(reason="feeds fp32r matmul"):
                nc.vector.tensor_copy(den_row, den_ps)
            bc_ps = ps_dv.tile([128, 512], f32, tag="dv")
            nc.tensor.matmul(bc_ps, ones_row, den_row, start=True, stop=True)
            bc_sb = work.tile([128, 512], f32, tag="bc_sb")
            with nc.allow_low_precision(reason="fp32r itr weights"):
                nc.vector.reciprocal(bc_sb, bc_ps)
                nc.vector.tensor_mul(itrt_r[:, c * 4:(c + 1) * 4, :],
                                     itr_ps, bc_sb)

        # ---- phase D: gates ----
        gcat = work.tile([128, NB, 384], f32, tag="gcat")
        for ib in range(NB):
            g_ps = ps_dv.tile([128, 384], f32, tag="dv")
            nc.tensor.matmul(g_ps, pt_r[:, ib, :], w_top, start=True, stop=False)
            nc.tensor.matmul(g_ps, itrt_r[:, ib, :], w_bot, start=False, stop=False)
            nc.tensor.matmul(g_ps, ones_row, bcat, start=False, stop=True)
            nc.vector.tensor_copy(gcat[:, ib, :], g_ps)

        z_t = work.tile([128, NB, 128], f32, tag="z_t")
        nc.scalar.activation(z_t, gcat[:, :, 0:128], Tanh)
        rf_t = work.tile([128, NB, 256], f32, tag="rf_t")
        nc.scalar.activation(rf_t, gcat[:, :, 128:384], Tanh, scale=0.5)
        # r = 0.5 + 0.5*tanh(0.5 x), f likewise
        rf_a = work.tile([128, NB, 256], f32, tag="rf_a")
        nc.gpsimd.tensor_scalar(rf_a, rf_t, 0.5, 0.5,
                                mybir.AluOpType.mult, mybir.AluOpType.add)

        m1 = work.tile([128, NB, 128], f32, tag="m1")
        nc.gpsimd.tensor_mul(m1, rf_a[:, :, 0:128], pn)       # r * P
        m2 = work.tile([128, NB, 128], f32, tag="m2")
        nc.vector.tensor_mul(m2, rf_a[:, :, 128:256], z_t)    # f * z
        out_sb = work.tile([128, NB, 128], f32, tag="out_sb")
        nc.vector.tensor_add(out_sb, m1, m2)

        nc.sync.dma_start(out=out[bi].rearrange("(t p) d -> p t d", p=128),
                          in_=out_sb)


_NC_CACHE = {}


def _get_nc():
    if "nc" not in _NC_CACHE:
        nc = bacc.Bacc(None)
        with tile.TileContext(nc) as tc:
            with ExitStack() as ctx:
                _body(nc, tc, ctx)
        nc.finalize()
        _NC_CACHE["nc"] = nc
    return _NC_CACHE["nc"]


def _run(inputs, **kw):
    nc = _get_nc()
    in_maps = []
    for c in range(NCORES):
        m = {
            "P": np.ascontiguousarray(inputs["P"][c * BPC:(c + 1) * BPC]),
            "w_itr_att": np.asarray(inputs["w_itr_att"]),
            "w1": np.asarray(inputs["w1"]),
            "w2": np.asarray(inputs["w2"]),
            "w3": np.asarray(inputs["w3"]),
            "b1": np.asarray(inputs["b1"]),
            "b2": np.asarray(inputs["b2"]),
            "b3": np.asarray(inputs["b3"]),
        }
        in_maps.append({k: np.asarray(v, dtype=np.float32) for k, v in m.items()})
    res = run_bass_kernel_spmd(nc, in_maps, core_ids=list(range(NCORES)), **kw)
    outp = np.concatenate([r["out"] for r in res.results], axis=0)
    return outp.astype(np.float32), res


def kernel(**inputs):
    out, _ = _run(inputs)
    return out


# revision 11
# speedup vs baseline: 1.3305x; 1.2940x over previous
"""Trainium2 Bass kernel for nn_Encoding (dense transformer block with
inter-attention + gated fusion), data-parallel over batch on 8 NeuronCores.

Reference math per batch b (P: [n, d], weights small):
  wa, wb, wc = split(w_itr_att)
  A[i,j]   = P[i].wb + P[j].wa + sum_d P[i,d]*wc[d]*P[j,d]
  SA       = softmax_j(A)
  itr      = SA @ P
  Pc       = [P, itr]
  z = tanh(Pc@w1+b1); r = sig(Pc@w2+b2); f = sig(Pc@w3+b3)
  out      = r*P + f*z

Key algebra used here:
  - exp(P[i].wb) cancels between softmax numerator and denominator, so the
    wb term is dropped entirely.
  - Scores are computed TRANSPOSED: At[j,i] = A[i,j], so the row term
    P[j].wa is a per-partition bias folded into the exp() activation and
    both the numerator matmul (P^T @ T) and denominator matmul
    (ones^T @ T) consume T=exp(At) in its natural layout with j on
    partitions (contraction axis) -- no transpose of the big [n,n] matrix.
  - sigmoid(x) = 0.5 + 0.5*tanh(0.5*x) keeps all activations within the
    exp/tanh ACT table set (no table switches).
Matmuls run in bf16 (separate LDWEIGHTS pipelines behind MATMUL, unlike
fp32/fp32r self-loading matmuls whose weight load serializes); accumulation
is fp32 in PSUM and all softmax/normalization arithmetic stays fp32.
"""
from contextlib import ExitStack

import numpy as np

import concourse.bass as bass
import concourse.mybir as mybir
import concourse.tile as tile
import concourse.tile_sem_assignment as tsa
from concourse import bacc
from concourse.bass_utils import run_bass_kernel_spmd
from concourse.masks import make_identity

# All HWDGE DMAs here are issued from the single SP sequencer (one physical
# FIFO ring -> in-order completion), so one completion semaphore suffices and
# keeps per-instruction sync-wait counts low.
tsa.NUM_HWDGE_SEMS = 1

B, N, D = 32, 1024, 128
NCORES = 8
BPC = B // NCORES          # batches per core
NB = N // 128              # 128-row blocks per batch
f32 = mybir.dt.float32
bf16 = mybir.dt.bfloat16
Exp = mybir.ActivationFunctionType.Exp
Tanh = mybir.ActivationFunctionType.Tanh


def _body(nc, tc, ctx):
    P = nc.dram_tensor("P", [BPC, N, D], f32, kind="ExternalInput")
    w_att = nc.dram_tensor("w_itr_att", [3 * D], f32, kind="ExternalInput")
    w1 = nc.dram_tensor("w1", [2 * D, D], f32, kind="ExternalInput")
    w2 = nc.dram_tensor("w2", [2 * D, D], f32, kind="ExternalInput")
    w3 = nc.dram_tensor("w3", [2 * D, D], f32, kind="ExternalInput")
    b1 = nc.dram_tensor("b1", [D], f32, kind="ExternalInput")
    b2 = nc.dram_tensor("b2", [D], f32, kind="ExternalInput")
    b3 = nc.dram_tensor("b3", [D], f32, kind="ExternalInput")
    out = nc.dram_tensor("out", [BPC, N, D], f32, kind="ExternalOutput")

    singles = ctx.enter_context(tc.tile_pool(name="singles", bufs=1))
    work = ctx.enter_context(tc.tile_pool(name="work", bufs=2))
    big = ctx.enter_context(tc.tile_pool(name="big", bufs=2))
    ps_at = ctx.enter_context(tc.tile_pool(name="ps_at", bufs=2, space="PSUM"))
    ps_itr = ctx.enter_context(tc.tile_pool(name="ps_itr", bufs=2, space="PSUM"))
    ps_dv = ctx.enter_context(tc.tile_pool(name="ps_dv", bufs=2, space="PSUM"))

    # ---- constants ----
    ident = singles.tile([128, 128], f32)
    make_identity(nc, ident)
    ident_h = singles.tile([128, 128], bf16)
    nc.vector.tensor_copy(ident_h, ident)

    ones_f = singles.tile([128, 1], f32)
    nc.vector.memset(ones_f, 1.0)
    ones_rf = singles.tile([1, 128], f32)
    nc.vector.memset(ones_rf, 1.0)
    ones_col = singles.tile([128, 1], bf16)    # lhsT for denominator matmul
    nc.vector.tensor_copy(ones_col, ones_f)
    ones_row = singles.tile([1, 128], bf16)    # lhsT for broadcast matmuls
    nc.vector.tensor_copy(ones_row, ones_rf)

    # wa / wc columns ([128,1]); wb is mathematically irrelevant (cancels).
    watt_sb = singles.tile([128, 3], f32)
    nc.sync.dma_start(out=watt_sb, in_=w_att.rearrange("(c p) -> p c", p=128))
    wa_col = singles.tile([128, 2], bf16)   # duplicated: small-N matmul quirk
    nc.vector.tensor_copy(wa_col[:, 0:1], watt_sb[:, 0:1])
    nc.vector.tensor_copy(wa_col[:, 1:2], watt_sb[:, 0:1])
    wc_col = singles.tile([128, 1], f32)
    nc.vector.tensor_copy(wc_col, watt_sb[:, 2:3])

    # Gate weights: Wtop = rows 0:128 of [w1|w2|w3], Wbot = rows 128:256.
    wstage = singles.tile([128, 2, 3, 128], f32)
    for gi, w in enumerate((w1, w2, w3)):
        nc.sync.dma_start(out=wstage[:, 0, gi, :], in_=w[0:128, :])
        nc.sync.dma_start(out=wstage[:, 1, gi, :], in_=w[128:256, :])
    w_top = singles.tile([128, 384], bf16)
    w_bot = singles.tile([128, 384], bf16)
    nc.vector.tensor_copy(w_top, wstage[:, 0, :, :])
    nc.vector.tensor_copy(w_bot, wstage[:, 1, :, :])

    bstage = singles.tile([1, 3, 128], f32)
    for gi, bvec in enumerate((b1, b2, b3)):
        nc.sync.dma_start(out=bstage[:, gi, :],
                          in_=bvec.rearrange("(o p) -> o p", o=1))
    bcat = singles.tile([1, 384], bf16)
    nc.vector.tensor_copy(bcat, bstage)

    for bi in range(BPC):
        # ---- phase A: load P, build P^T (via PE transpose), Pwc^T, v ----
        pn = work.tile([128, NB, 128], f32, tag="pn")
        nc.sync.dma_start(out=pn, in_=P[bi].rearrange("(t p) d -> p t d", p=128))
        pn_h = work.tile([128, NB, 128], bf16, tag="pn_h")
        nc.gpsimd.tensor_copy(pn_h, pn)

        pt_h = work.tile([128, NB, 128], bf16, tag="pt_h")   # [d, n]
        for half in range(2):
            tp_ps = ps_itr.tile([128, 512], bf16, tag="itr")
            for q in range(4):
                jb = half * 4 + q
                nc.tensor.transpose(tp_ps[:, q * 128:(q + 1) * 128],
                                    pn_h[:, jb, :], ident_h)
            nc.vector.tensor_copy(pt_h[:, half * 4:(half + 1) * 4, :], tp_ps)

        pwct_h = work.tile([128, NB, 128], bf16, tag="pwct_h")  # wc[d]*P^T
        nc.vector.tensor_scalar_mul(pwct_h, pt_h, wc_col)

        # v[j] = P[j].wa computed per j-block into [128, NB, 2] (col 0 used)
        v_ps = ps_dv.tile([128, NB, 2], f32, tag="dv")
        for jb in range(NB):
            nc.tensor.matmul(v_ps[:, jb, :], pt_h[:, jb, :], wa_col,
                             start=True, stop=True)
        v_sb = work.tile([128, NB, 2], f32, tag="v_sb")
        nc.vector.tensor_copy(v_sb, v_ps)

        # ---- phase B: scores At[j,i] + exp -> T (bf16) ----
        st = big.tile([128, NB, N], bf16, tag="st")
        for jb in range(NB):
            at_ps = ps_at.tile([128, 1024], f32, tag="at")
            nc.tensor.matmul(at_ps[:, 0:512], pt_h[:, jb, :],
                             pwct_h[:, 0:4, :], start=True, stop=True)
            nc.tensor.matmul(at_ps[:, 512:1024], pt_h[:, jb, :],
                             pwct_h[:, 4:8, :], start=True, stop=True)
            nc.scalar.activation(st[:, jb, :], at_ps, Exp,
                                 bias=v_sb[:, jb, 0:1])

        # ---- phase C: numerator/denominator + normalize ----
        itrt_h = work.tile([128, NB, 128], bf16, tag="itrt_h")  # itr^T [d, n]
        for c in range(2):
            cs = slice(c * 512, (c + 1) * 512)
            itr_ps = ps_itr.tile([128, 512], f32, tag="itr")
            den_ps = ps_dv.tile([1, 512], f32, tag="dv")
            for jb in range(NB):
                nc.tensor.matmul(itr_ps, pn_h[:, jb, :], st[:, jb, cs],
                                 start=(jb == 0), stop=(jb == NB - 1))
            for jb in range(NB):
                nc.tensor.matmul(den_ps, ones_col, st[:, jb, cs],
                                 start=(jb == 0), stop=(jb == NB - 1))
            # broadcast raw denominator to all partitions via ones x den,
            # then fast reciprocal on [128,512] and multiply.
            den_row = work.tile([1, 512], bf16, tag="den_row")
            nc.vector.tensor_copy(den_row, den_ps)
            bc_ps = ps_dv.tile([128, 512], f32, tag="dv")
            nc.tensor.matmul(bc_ps, ones_row, den_row, start=True, stop=True)
            bc_sb = work.tile([128, 512], f32, tag="bc_sb")
            nc.vector.tensor_copy(bc_sb, bc_ps)
            recip_sb = work.tile([128, 512], f32, tag="recip_sb")
            nc.vector.reciprocal_approx_fast(recip_sb, bc_sb)
            with nc.allow_low_precision(reason="bf16 itr weights"):
                nc.vector.tensor_mul(itrt_h[:, c * 4:(c + 1) * 4, :],
                                     itr_ps, recip_sb)

        # ---- phase D: gates ----
        gcat = work.tile([128, NB, 384], f32, tag="gcat")
        for ib in range(NB):
            g_ps = ps_dv.tile([128, 384], f32, tag="dv")
            nc.tensor.matmul(g_ps, pt_h[:, ib, :], w_top, start=True, stop=False)
            nc.tensor.matmul(g_ps, itrt_h[:, ib, :], w_bot, start=False, stop=False)
            nc.tensor.matmul(g_ps, ones_row, bcat, start=False, stop=True)
            nc.vector.tensor_copy(gcat[:, ib, :], g_ps)

        z_t = work.tile([128, NB, 128], f32, tag="z_t")
        nc.scalar.activation(z_t, gcat[:, :, 0:128], Tanh)
        rf_t = work.tile([128, NB, 256], f32, tag="rf_t")
        nc.scalar.activation(rf_t, gcat[:, :, 128:384], Tanh, scale=0.5)
        # r = 0.5 + 0.5*tanh(0.5 x), f likewise
        rf_a = work.tile([128, NB, 256], f32, tag="rf_a")
        nc.gpsimd.tensor_scalar(rf_a, rf_t, 0.5, 0.5,
                                mybir.AluOpType.mult, mybir.AluOpType.add)

        m1 = work.tile([128, NB, 128], f32, tag="m1")
        nc.gpsimd.tensor_mul(m1, rf_a[:, :, 0:128], pn)       # r * P
        m2 = work.tile([128, NB, 128], f32, tag="m2")
        nc.vector.tensor_mul(m2, rf_a[:, :, 128:256], z_t)    # f * z
        out_sb = work.tile([128, NB, 128], f32, tag="out_sb")
        nc.vector.tensor_add(out_sb, m1, m2)

        nc.sync.dma_start(out=out[bi].rearrange("(t p) d -> p t d", p=128),
                          in_=out_sb)


_NC_CACHE = {}


def _get_nc():
    if "nc" not in _NC_CACHE:
        nc = bacc.Bacc(None)
        with tile.TileContext(nc) as tc:
            with ExitStack() as ctx:
                _body(nc, tc, ctx)
        nc.finalize()
        _NC_CACHE["nc"] = nc
    return _NC_CACHE["nc"]


def _run(inputs, **kw):
    nc = _get_nc()
    in_maps = []
    for c in range(NCORES):
        m = {
            "P": np.ascontiguousarray(inputs["P"][c * BPC:(c + 1) * BPC]),
            "w_itr_att": np.asarray(inputs["w_itr_att"]),
            "w1": np.asarray(inputs["w1"]),
            "w2": np.asarray(inputs["w2"]),
            "w3": np.asarray(inputs["w3"]),
            "b1": np.asarray(inputs["b1"]),
            "b2": np.asarray(inputs["b2"]),
            "b3": np.asarray(inputs["b3"]),
        }
        in_maps.append({k: np.asarray(v, dtype=np.float32) for k, v in m.items()})
    res = run_bass_kernel_spmd(nc, in_maps, core_ids=list(range(NCORES)), **kw)
    outp = np.concatenate([r["out"] for r in res.results], axis=0)
    return outp.astype(np.float32), res


def kernel(**inputs):
    out, _ = _run(inputs)
    return out
